# revision 22
# baseline (speedup 1.0000x reference)
"""Contrastive-loss kernel v6: host pre-gathered pairs, PE reductions,
arrival-matched engine schedule, optional per-chunk fp8 streaming.

Reference semantics (B=4, N=4096, D=128, T=0.1):
    u = emb / max(||emb||, 1e-12)
    pos_sim[b,n] = dot(u[b,n], u[b, pos_idx[b,n]]) / T
    loss = mean(softplus(-pos_sim)) + mean(softplus(neg_sim))

Sharding: each of 8 cores takes half the rows ("slots") of one batch
element; the host ships per slot the own row and both partner rows
(index-select + dtype cast only; all arithmetic on device) in
[D=128, slot] layout, chunk-blocked for linear DMA. Chunks may be bf16
or fp8e4 (halves DMA bytes; squares on ACT/Pool are dtype-blind,
products on DVE drop from 2x to 1x).

Per chunk: squares as one merged [128, 3C] self-product split across
DVE/ACT/Pool, products as a single broadcast tensor_tensor
[128, 2, C]. All per-slot D-reductions are 1-column PE matmuls
(stationary = 128-slot block; the ones-vector folds -+1/T).

z-chain per group ('w' style):  u = ln(ssq); w = u_pair + u_own;
r = exp(-0.5 w); z = dots * r; acc += ln(exp(z) + 1); ('m' style)
m = ssq_pair * ssq_own; r = exp(-0.5 ln m); ... Ln/Exp/Square pinned
to one act table.

Output: [128, NGROUP] f32 partial-sum tile; host sums / (B*N).
"""

import numpy as np

B, N, D = 4, 4096, 128
NCORES = 8
HALF = N // 2            # slots per core
TEMP = 0.1

CFG = dict(
    chunks=(640, 384, 512, 384, 128),
    groups=(0, 0, 0, 0, 1),
    dtype=("fp8", "bf16", "bf16", "bf16", "bf16"),
    prod=("dve", "dve", "dve", "dve", "dve"),
    sq=(
        (("pool", 0, 512), ("act", 512, 1920)),
        (("pool", 0, 640), ("act", 640, 1152)),
        (("pool", 0, 128), ("dve", 128, 512), ("act", 512, 1536)),
        (("pool", 0, 384), ("dve", 384, 768), ("act", 768, 1152)),
        (("act", 0, 384),),
    ),
    zeng=("pool", "pool"),
    zstyle="w",
    pe_split=False,
    ztail=True,
    out="plain",
    out_eng="sp",
    const_patch="dve",
)

_PROG = None
_PROG_CFG = None


def _pin_act_table(table_name="natural_log_exp_and_others"):
    import functools
    import concourse.hw_specs as hw_specs
    import concourse.bacc as bacc
    import concourse.mybir as mybir

    if getattr(_pin_act_table, "_done", False):
        return
    orig = hw_specs.get_activation_tables
    AF = mybir.ActivationFunctionType
    pinned = {AF.Square, AF.Ln, AF.Exp}

    @functools.cache
    def patched(arch):
        return {k: (v if k == table_name else v - pinned)
                for k, v in orig(arch).items()}

    hw_specs.get_activation_tables = patched
    bacc.get_activation_tables = patched
    _pin_act_table._done = True


def _chunk_meta(cfg):
    """byte offsets of each chunk in the uint8 data blob"""
    offs = []
    off = 0
    for C, dt in zip(cfg["chunks"], cfg["dtype"]):
        nb = 3 * C * (2 if dt == "bf16" else 1)
        offs.append((off, nb))
        off += nb
    return offs, off


def _build_program(cfg=None):
    import concourse.bacc as bacc
    import concourse.bass as bass_mod
    import concourse.tile as tile
    import concourse.mybir as mybir

    cfg = dict(CFG if cfg is None else cfg)
    _pin_act_table()

    f32 = mybir.dt.float32
    bf16 = mybir.dt.bfloat16
    fp8 = mybir.dt.float8e4
    u8 = mybir.dt.uint8
    mult = mybir.AluOpType.mult
    add = mybir.AluOpType.add
    AF = mybir.ActivationFunctionType

    CHUNKS = cfg["chunks"]
    GRP = cfg["groups"]
    NCH = len(CHUNKS)
    NGRP = max(GRP) + 1
    assert sum(CHUNKS) == HALF
    g_nb = [sum(CHUNKS[k] for k in range(NCH) if GRP[k] == g) // 128
            for g in range(NGRP)]
    g_boff = []
    off = [0] * NGRP
    for k in range(NCH):
        g = GRP[k]
        g_boff.append(off[g])
        off[g] += CHUNKS[k] // 128
    c_boff, totbytes = _chunk_meta(cfg)

    orig_memset = bass_mod.BassGpSimd.memset
    if cfg["const_patch"] == "dve":
        # run the framework's preamble const memsets on DVE so the Pool
        # engine reaches the entry barrier sooner (first DMA issues earlier)
        bass_mod.BassGpSimd.memset = lambda self, ap, val: self.bass.vector.memset(ap, val)
    elif cfg["const_patch"]:
        bass_mod.BassGpSimd.memset = lambda self, ap, val: None
    try:
        nc = bacc.Bacc("TRN2", target_bir_lowering=False)
    finally:
        bass_mod.BassGpSimd.memset = orig_memset

    data = nc.dram_tensor("data", [128, totbytes], u8, kind="ExternalInput")
    trig = cfg["out"] == "trig"
    i16 = mybir.dt.int16
    out = nc.dram_tensor("partial", [128, 64 if trig else NGRP], f32,
                         kind="ExternalOutput")

    if trig:
        # raw (non-tile-pool) tensors so the scatter prep can live in the
        # main block, outside Tile's DMASW bookkeeping
        accT = nc.alloc_sbuf_tensor("accT", [128, 64], f32)
        idxT = nc.alloc_sbuf_tensor("idxT", [16, 8], i16)
        zeroT = nc.alloc_sbuf_tensor("zeroT", [128, 64], f32)
        dma_sem = nc.alloc_semaphore("odma")
        # idxs are consumed by desc-gen at prep time -> init first (main block)
        nc.gpsimd.iota(idxT.ap(), pattern=[[16, 8]], base=0, channel_multiplier=1)
        nc.gpsimd.dma_scatter_add(out[:], accT.ap().unsqueeze(1), idxT.ap(),
                                  128, 128, 64, prepare_only=True, sem=dma_sem)

    with tile.TileContext(nc) as tc:
        with tc.tile_pool(name="sb", bufs=1) as pool, \
             tc.tile_pool(name="ps", bufs=1, space="PSUM") as psum:

            ones = pool.tile([128, 3], bf16, tag="ones")   # [+1, -1/T, +1/T]
            nc.vector.memset(ones[:, 0:1], 1.0)
            nc.vector.memset(ones[:, 1:2], -1.0 / TEMP)
            nc.vector.memset(ones[:, 2:3], 1.0 / TEMP)
            if cfg["const_patch"] and cfg["const_patch"] != "dve":
                czero = pool.tile([128, 1], f32, tag="czero")
                nc.vector.memset(czero[:], 0.0)
                cone = pool.tile([128, 1], f32, tag="cone")
                nc.vector.memset(cone[:], 1.0)
                zb = {"bias": czero[:]}
                ob = {"bias": cone[:]}
            else:
                zb = {}
                ob = {"bias": 1.0}

            # input chunks: all DMAs issued up-front on SP
            chunks = []
            for k, C in enumerate(CHUNKS):
                dt = bf16 if cfg["dtype"][k] == "bf16" else fp8
                t = pool.tile([128, 3 * C], dt, tag=f"d{k}", name=f"d{k}")
                bo, nb_ = c_boff[k]
                nc.sync.dma_start(out=t[:], in_=data[:, bo:bo + nb_].bitcast(dt))
                chunks.append(t)

            # group psum layout (nb blocks): [ssq_gp | ssq_gn | ssq_o | dot_p | dot_n]
            q_t = []
            for g in range(NGRP):
                qg = psum.tile([128, 5 * g_nb[g]], f32, tag=f"q{g}", name=f"q{g}")
                q_t.append(qg)
            if trig:
                acc = accT.ap()
                nc.vector.memset(acc[:, :], 0.0)
                nc.vector.memset(zeroT.ap()[:, :], 0.0)
                # zero the scatter-add target once the input stream is done
                nc.sync.dma_start(out=out[:], in_=zeroT.ap())
            else:
                acc = pool.tile([128, NGRP], f32, tag="acc")

            def ew(eng, out_ap, in0, in1, op):
                if eng == "dve":
                    nc.vector.tensor_tensor(out=out_ap, in0=in0, in1=in1, op=op)
                elif eng == "pool":
                    nc.gpsimd.tensor_tensor(out=out_ap, in0=in0, in1=in1, op=op)
                else:
                    raise ValueError(eng)

            sq_t = [None] * NCH
            pr_t = [None] * NCH

            def emit_chunk(k, C, part):
                g, nb = GRP[k], g_nb[GRP[k]]
                bo = g_boff[k]
                q = q_t[g]
                d = chunks[k]
                if part in ("all", "ew"):
                    sq = pool.tile([128, 3 * C], bf16, tag=f"sq{k}", name=f"sq{k}")
                    sq_t[k] = sq
                    for e, s, t_ in cfg["sq"][k]:
                        if e == "act":
                            nc.scalar.activation(sq[:, s:t_], d[:, s:t_], AF.Square, **zb)
                        else:
                            ew(e, sq[:, s:t_], d[:, s:t_], d[:, s:t_], mult)
                    pr = pool.tile([128, 2 * C], bf16, tag=f"pr{k}", name=f"pr{k}")
                    pr_t[k] = pr
                    if cfg["prod"][k] == "dve":
                        o_b = d[:, 0:C].unsqueeze(1).to_broadcast([128, 2, C])
                        nc.vector.tensor_tensor(out=pr[:], in0=o_b, in1=d[:, C:3 * C], op=mult)
                    else:
                        nc.gpsimd.tensor_tensor(out=pr[:, 0:C], in0=d[:, 0:C],
                                                in1=d[:, C:2 * C], op=mult)
                        nc.gpsimd.tensor_tensor(out=pr[:, C:2 * C], in0=d[:, 0:C],
                                                in1=d[:, 2 * C:3 * C], op=mult)
                sq, pr = sq_t[k], pr_t[k]
                if part in ("all", "ssq"):
                    for j in range(C // 128):
                        col = bo + j
                        for base, t_ in ((0, sq[:, C + j * 128:C + (j + 1) * 128]),
                                         (nb, sq[:, 2 * C + j * 128:2 * C + (j + 1) * 128]),
                                         (2 * nb, sq[:, j * 128:(j + 1) * 128])):
                            nc.tensor.matmul(q[:, base + col:base + col + 1],
                                             t_, ones[:, 0:1], start=True, stop=True)
                if part in ("all", "dots"):
                    for j in range(C // 128):
                        col = bo + j
                        for base, t_, rhs in ((3 * nb, pr[:, j * 128:(j + 1) * 128], 1),
                                              (4 * nb, pr[:, C + j * 128:C + (j + 1) * 128], 2)):
                            nc.tensor.matmul(q[:, base + col:base + col + 1],
                                             t_, ones[:, rhs:rhs + 1],
                                             start=True, stop=True)

            def emit_z(g):
                nb = g_nb[g]
                q = q_t[g]
                ze = cfg["zeng"][g]
                if nb == 1 and cfg.get("ztail", True):
                    # single-block group: whole chain on ACT via per-partition
                    # scale/bias APs -- no cross-engine hops.
                    u = pool.tile([128, 3], f32, tag=f"u{g}", name=f"u{g}")
                    nc.scalar.activation(u[:], q[:, 0:3], AF.Ln, **zb)
                    v = pool.tile([128, 1], f32, tag=f"v{g}", name=f"v{g}")
                    nc.scalar.activation(v[:], u[:, 2:3], AF.Copy, scale=-0.5)
                    r = pool.tile([128, 2], f32, tag=f"r{g}", name=f"r{g}")
                    nc.scalar.activation(r[:], u[:, 0:2], AF.Exp, scale=-0.5,
                                         bias=v[:, 0:1])
                    e = pool.tile([128, 2], f32, tag=f"e{g}", name=f"e{g}")
                    nc.scalar.activation(e[:, 0:1], q[:, 3:4], AF.Exp,
                                         scale=r[:, 0:1], **zb)
                    nc.scalar.activation(e[:, 1:2], q[:, 4:5], AF.Exp,
                                         scale=r[:, 1:2], **zb)
                    sp = pool.tile([128, 2], f32, tag=f"sp{g}", name=f"sp{g}")
                    nc.scalar.activation(sp[:], e[:], AF.Ln,
                                         accum_out=acc[:, g:g + 1], **ob)
                    return
                if cfg["zstyle"] == "m":
                    # q is PSUM: gpsimd cannot read PSUM, so these stay on DVE
                    m = pool.tile([128, 2 * nb], f32, tag=f"m{g}", name=f"m{g}")
                    if nb == 1:
                        nc.vector.tensor_scalar(out=m[:], in0=q[:, 0:2],
                                                scalar1=q[:, 2:3], scalar2=None, op0=mult)
                    else:
                        ew("dve", m[:, 0:nb], q[:, 0:nb], q[:, 2 * nb:3 * nb], mult)
                        ew("dve", m[:, nb:2 * nb], q[:, nb:2 * nb], q[:, 2 * nb:3 * nb], mult)
                    lm = pool.tile([128, 2 * nb], f32, tag=f"lm{g}", name=f"lm{g}")
                    nc.scalar.activation(lm[:], m[:], AF.Ln, **zb)
                    r = pool.tile([128, 2 * nb], f32, tag=f"r{g}", name=f"r{g}")
                    nc.scalar.activation(r[:], lm[:], AF.Exp, scale=-0.5, **zb)
                else:
                    u = pool.tile([128, 3 * nb], f32, tag=f"u{g}", name=f"u{g}")
                    nc.scalar.activation(u[:], q[:, 0:3 * nb], AF.Ln, **zb)
                    w = pool.tile([128, 2 * nb], f32, tag=f"w{g}", name=f"w{g}")
                    if nb == 1:
                        nc.vector.tensor_scalar(out=w[:], in0=u[:, 0:2],
                                                scalar1=u[:, 2:3], scalar2=None, op0=add)
                    else:
                        ew(ze, w[:, 0:nb], u[:, 0:nb], u[:, 2 * nb:3 * nb], add)
                        ew(ze, w[:, nb:2 * nb], u[:, nb:2 * nb], u[:, 2 * nb:3 * nb], add)
                    r = pool.tile([128, 2 * nb], f32, tag=f"r{g}", name=f"r{g}")
                    nc.scalar.activation(r[:], w[:], AF.Exp, scale=-0.5, **zb)
                z = pool.tile([128, 2 * nb], f32, tag=f"z{g}", name=f"z{g}")
                ew("dve", z[:], q[:, 3 * nb:5 * nb], r[:], mult)
                e = pool.tile([128, 2 * nb], f32, tag=f"e{g}", name=f"e{g}")
                nc.scalar.activation(e[:], z[:], AF.Exp, **zb)
                sp = pool.tile([128, 2 * nb], f32, tag=f"sp{g}", name=f"sp{g}")
                nc.scalar.activation(sp[:], e[:], AF.Ln,
                                     accum_out=acc[:, g:g + 1], **ob)

            last_of_group = {}
            for k in range(NCH):
                last_of_group[GRP[k]] = k
            ztail_first = cfg.get("ztail_first", False)
            for k in range(NCH):
                if cfg["pe_split"]:
                    emit_chunk(k, CHUNKS[k], "ew")
                    emit_chunk(k, CHUNKS[k], "ssq")
                    emit_chunk(k, CHUNKS[k], "dots")
                else:
                    emit_chunk(k, CHUNKS[k], "all")
                if cfg.get("diag_no_z") or ztail_first:
                    continue
                for g in range(NGRP):
                    if last_of_group[g] == k:
                        emit_z(g)
            if ztail_first and not cfg.get("diag_no_z"):
                # emit 1-block (all-ACT) groups first so their chains slot
                # into the big groups' stall gaps on the ACT queue
                order = sorted(range(NGRP), key=lambda g: (g_nb[g] != 1,))
                for g in order:
                    emit_z(g)

            if cfg.get("diag_no_z"):
                nc.vector.memset(acc[:], 0.0)
            out_eng = {"sp": nc.sync, "act": nc.scalar,
                       "dve": nc.vector, "pool": nc.gpsimd}[cfg.get("out_eng", "sp")]
            if trig:
                pass  # output fired post-context via trigger_dma
            elif not cfg.get("diag_no_out"):
                out_eng.dma_start(out=out[:], in_=acc[:])
            else:
                out_eng.dma_start(out=out[:], in_=ones[:, 0:NGRP].bitcast(f32))

    if trig:
        # ordered after the end-of-context all-engine barrier, which
        # guarantees the accumulator writes have completed
        nc.gpsimd.trigger_dma(count=1)
        nc.gpsimd.wait_ge(dma_sem, 16)

    nc.compile()
    return nc


def _get_program():
    global _PROG, _PROG_CFG
    if _PROG is None or _PROG_CFG != repr(CFG):
        _PROG = _build_program()
        _PROG_CFG = repr(CFG)
    return _PROG


def _shard_inputs(embeddings, positive_pairs, negative_pairs, cfg=None):
    import ml_dtypes
    import concourse.mybir as mybir

    cfg = CFG if cfg is None else cfg
    CHUNKS = cfg["chunks"]
    fp8_np = mybir.dt.np(mybir.dt.float8e4)
    emb = np.asarray(embeddings, dtype=np.float32)
    emb_bf = emb.astype(ml_dtypes.bfloat16)
    pos = np.asarray(positive_pairs).reshape(B, N)
    neg = np.asarray(negative_pairs).reshape(B, N)

    in_maps = []
    for c in range(NCORES):
        b, h = divmod(c, 2)
        base = h * HALF
        E = emb_bf[b]
        o_t = E[base:base + HALF].T          # [128, HALF]
        gp_t = E[pos[b, base:base + HALF]].T
        gn_t = E[neg[b, base:base + HALF]].T
        blocks = []
        for k, C in enumerate(CHUNKS):
            s = slice(sum(CHUNKS[:k]), sum(CHUNKS[:k]) + C)
            blk = np.ascontiguousarray(
                np.concatenate([o_t[:, s], gp_t[:, s], gn_t[:, s]], axis=1))
            if cfg["dtype"][k] == "fp8":
                blk = blk.astype(np.float32).astype(fp8_np)
            blocks.append(blk.view(np.uint8))
        in_maps.append({"data": np.ascontiguousarray(np.concatenate(blocks, axis=1))})
    return in_maps


def kernel(embeddings, positive_pairs, negative_pairs):
    from concourse.bass_utils import run_bass_kernel_spmd

    nc = _get_program()
    in_maps = _shard_inputs(embeddings, positive_pairs, negative_pairs)
    res = run_bass_kernel_spmd(nc, in_maps, core_ids=list(range(NCORES)))
    ngrp = max(CFG["groups"]) + 1
    total = sum(r["partial"][:, :ngrp].astype(np.float64).sum() for r in res.results)
    return np.float32(total / (B * N))
